# revision 20
# baseline (speedup 1.0000x reference)
"""Bahdanau-style attention kernel for Trainium2 (8 NeuronCores, SPMD).

Computation (per batch element b):
    q[b]      = hidden[b] @ W1.T                          # [H], W1 = W[:, :2H]
    pre[b,s]  = enc[b,s] @ W2.T + q[b] + bias             # [S, H], W2 = W[:, 2H:]
    energy    = tanh(pre)                                 # [S, H]
    scores    = energy @ v                                # [S]
    attn      = softmax(scores)                           # [S]
    ctx[b]    = enc[b].T @ attn                           # [2H]

Sharding: data-parallel over batch, 4 batches per core, W/b/v replicated.

Layout strategy per batch (per core):
  - enc loaded naturally as 16 tiles e_nat[i] = [s=128, f=1024] (s on partitions).
  - PE transposes 128x128 blocks -> eT tiles [f=128, s=512] for the main matmul
    (contraction over f needs f on partitions).
  - MM1: psum[h=128, s=512] = W2T[f,h].T @ eT[f,s], accumulated over 8 f-chunks.
  - ACT applies tanh with per-partition bias (q[b]+bias)[h-chunk] -> energyT sbuf.
  - MM2: duplicated score column [s=128,2] = energyT[h,s-chunk].T @ v2[h,2],
    accumulated over h-chunks (f32r needs lhsT free dim 128 and even moving N).
  - softmax without max-subtraction (scores bounded by sum|v| ~ 20; exp safe in
    f32): p2 = exp(scores) duplicated, accum_out gives per-partition sums,
    Z broadcast to all partitions via ones matmul.
  - MM3: ctx column pair [f=128,2] = e_nat[i][:,fc].T @ p2[:,i,:], accumulated
    over s-chunks with a single start=True (per-byte has_written lets the 8
    fc-groups share one PSUM bank when only the first matmul starts).
  - scale by 1/Z on DVE, DMA out (column layout -> strided store).

float32r (reduced-precision fp32 matmul, ~4x PE throughput, measured end-to-end
rel err ~8e-5) is used on all heavy matmul paths when fast=True.
"""

import contextlib
import sys

sys.path.insert(0, "/opt/trn_rl_repo")

import numpy as np

import concourse.bass as bass
import concourse.tile as tile
from concourse import bacc, mybir
from concourse.bass_utils import run_bass_kernel_spmd
from concourse.masks import make_identity

F32 = mybir.dt.float32

N_CORES = 8
B = 32
B_LOC = B // N_CORES  # 4 batches per core
S = 2048
H = 512
F = 1024  # 2H = encoder feature dim
NS = S // 128  # 16 s-chunks
NF = F // 128  # 8 f-chunks
NH = H // 128  # 4 h-chunks
NSG = S // 512  # 4 s-groups of 512


def _build(fast=True, reps=1, ablate=()):
    MDT = mybir.dt.float32r if fast else F32
    nc = bacc.Bacc(None, target_bir_lowering=False)

    hid_d = nc.dram_tensor("hidden", [B_LOC, 2 * H], F32, kind="ExternalInput")
    enc_d = nc.dram_tensor("enc", [B_LOC, S, F], MDT, kind="ExternalInput")
    # W.T host-prepared: [2048, 512]; rows 0:1024 = W1.T, 1024:2048 = W2.T
    wt_d = nc.dram_tensor("wt", [4 * H, H], MDT, kind="ExternalInput")
    bias_d = nc.dram_tensor("bias_in", [H], F32, kind="ExternalInput")
    v_d = nc.dram_tensor("v_in", [H], F32, kind="ExternalInput")
    out_d = nc.dram_tensor("out", [B_LOC, F], F32, kind="ExternalOutput")

    with tile.TileContext(nc) as tc:
        with (
            tc.tile_pool(name="singles", bufs=1) as singles,
            tc.tile_pool(name="enat", bufs=22) as enat_pool,
            tc.tile_pool(name="et", bufs=16) as et_pool,
            tc.tile_pool(name="energy", bufs=12) as energy_pool,
            tc.tile_pool(name="small", bufs=4) as small_pool,
            tc.tile_pool(name="ps_et", bufs=2, space="PSUM") as ps_et,
            tc.tile_pool(name="ps_mm1", bufs=4, space="PSUM") as ps_mm1,
            tc.tile_pool(name="ps_small", bufs=2, space="PSUM") as ps_small,
        ):
            # ---------------- prologue (once per core) ----------------
            ident = singles.tile([128, 128], F32)
            make_identity(nc, ident)
            if fast:
                ident_r = singles.tile([128, 128], MDT)
                nc.vector.tensor_copy(ident_r, ident)
            else:
                ident_r = ident
            ones_bcast = singles.tile([128, 128], F32)
            nc.vector.memset(ones_bcast, 1.0)

            wt_sb = singles.tile([128, 4 * H // 128, H], MDT)
            nc.sync.dma_start(
                out=wt_sb, in_=wt_d.rearrange("(c p) n -> p c n", p=128)
            )
            v_cols = singles.tile([128, NH], F32)
            nc.sync.dma_start(out=v_cols, in_=v_d.rearrange("(c p) -> p c", p=128))
            bias_cols = singles.tile([128, NH], F32)
            nc.sync.dma_start(
                out=bias_cols, in_=bias_d.rearrange("(c p) -> p c", p=128)
            )
            # duplicated v columns for f32r MM2 (moving operand must be even-width)
            v2 = singles.tile([128, NH, 2], MDT)
            for hc in range(NH):
                nc.scalar.activation(
                    v2[:, hc, :],
                    v_cols[:, hc : hc + 1].broadcast_to((128, 2)),
                    mybir.ActivationFunctionType.Copy,
                )

            # hidden -> hT: column (c*B_LOC + b) = hidden[b, c*128:(c+1)*128]
            hid_sb = singles.tile([B_LOC, 2 * H], F32)
            nc.sync.dma_start(out=hid_sb, in_=hid_d[:, :])
            hT_ps = ps_et.tile([128, 2 * H // 128 * B_LOC], F32, tag="et")
            for c in range(2 * H // 128):
                nc.tensor.transpose(
                    hT_ps[:, c * B_LOC : (c + 1) * B_LOC],
                    hid_sb[:, c * 128 : (c + 1) * 128],
                    ident[:B_LOC, :B_LOC],
                )
            hT_sb = singles.tile([128, 2 * H // 128 * B_LOC], F32)
            nc.scalar.activation(hT_sb, hT_ps, mybir.ActivationFunctionType.Copy)

            # q = hidden @ W1.T (plain fp32, tiny M=4)
            q_ps = ps_mm1.tile([B_LOC, H], F32, tag="mm1")
            for c in range(2 * H // 128):
                nc.tensor.matmul(
                    q_ps,
                    hT_sb[:, c * B_LOC : (c + 1) * B_LOC],
                    wt_sb[:, c, :].bitcast(F32),
                    start=(c == 0),
                    stop=(c == 2 * H // 128 - 1),
                )
            q_sb = singles.tile([B_LOC, H], F32)
            nc.scalar.activation(q_sb, q_ps, mybir.ActivationFunctionType.Copy)

            # qT columns; bias_q[:, hc*B_LOC + b] = q[b, hc-chunk] + bias[hc-chunk]
            qT_ps = ps_et.tile([128, NH * B_LOC], F32, tag="et")
            for hc in range(NH):
                nc.tensor.transpose(
                    qT_ps[:, hc * B_LOC : (hc + 1) * B_LOC],
                    q_sb[:, hc * 128 : (hc + 1) * 128],
                    ident[:B_LOC, :B_LOC],
                )
            bias_q = singles.tile([128, NH * B_LOC], F32)
            for hc in range(NH):
                nc.vector.tensor_scalar_add(
                    bias_q[:, hc * B_LOC : (hc + 1) * B_LOC],
                    qT_ps[:, hc * B_LOC : (hc + 1) * B_LOC],
                    bias_cols[:, hc : hc + 1],
                )

            # ------------- software-pipelined per-s-group stream -------------
            # Global groups g = (b, sg). PE is in-order, so dependent matmuls
            # are emitted LATE: MM2(g) one step after its tanh, MM3(g) two
            # steps after its exp. While ACT computes exp(g), PE is busy with
            # transposes/MM1 of g+1 and MM3 of g-1 -- no cross-engine stalls.
            groups = [(b, sg) for b in range(B_LOC) for sg in range(NSG)]
            NG = len(groups)
            state = {}  # per-batch tiles
            gstate = {}  # per-group tiles

            def stage0(g):  # DMA + transposes + MM1 + tanh issue
                b, sg = groups[g]
                if sg == 0:
                    sps = ps_small.tile([128, 128], F32, tag="sps")
                    state[b] = dict(
                        sps=sps,
                        scores2=sps[:, 0 : 2 * NS].rearrange(
                            "p (i two) -> p i two", two=2
                        ),
                        z2=sps[:, 2 * NS : 2 * NS + 1],
                        p2=small_pool.tile([128, NS, 2], MDT, tag="p", name="p2"),
                        colsum4=small_pool.tile(
                            [128, NSG], F32, tag="colsum", name="colsum4"
                        ),
                        ctx_acc=small_pool.tile(
                            [128, NF], F32, tag="ctxacc", name="ctx_acc"
                        ),
                    )
                st = state[b]
                e_nat = []
                for j in range(4):
                    i = sg * 4 + j
                    t = enat_pool.tile([128, F], MDT, tag="enat")
                    if "dma" not in ablate:
                        nc.sync.dma_start(
                            out=t, in_=enc_d[b, i * 128 : (i + 1) * 128, :]
                        )
                    e_nat.append(t)

                if "trans" in ablate:
                    et_sb = [wt_sb[:, fc, :] for fc in range(NF)]
                else:
                    et_sb = []
                    for fc in range(NF):
                        et_ps_t = ps_et.tile([128, 512], MDT, tag="et")
                        for j in range(4):
                            nc.tensor.transpose(
                                et_ps_t[:, j * 128 : (j + 1) * 128],
                                e_nat[j][:, fc * 128 : (fc + 1) * 128],
                                ident_r,
                            )
                        t = et_pool.tile([128, 512], MDT, tag="et")
                        nc.vector.tensor_copy(t, et_ps_t)
                        et_sb.append(t)

                energyT = []
                for hc in range(NH):
                    mm1 = ps_mm1.tile([128, 512], F32, tag="mm1")
                    for fc in range(NF):
                        nc.tensor.matmul(
                            mm1,
                            wt_sb[:, NF + fc, hc * 128 : (hc + 1) * 128],
                            et_sb[fc],
                            start=(fc == 0),
                            stop=(fc == NF - 1),
                        )
                    en = energy_pool.tile([128, 512], MDT, tag="energy")
                    nc.scalar.activation(
                        en,
                        mm1,
                        mybir.ActivationFunctionType.Tanh,
                        bias=bias_q[:, hc * B_LOC + b : hc * B_LOC + b + 1],
                    )
                    energyT.append(en)
                gstate[g] = dict(e_nat=e_nat, energyT=energyT)

            def stage1(g):  # MM2 + exp issue (tanh of g finished during g+1's MM1)
                b, sg = groups[g]
                st = state[b]
                energyT = gstate[g]["energyT"]
                for j in range(4):
                    i = sg * 4 + j
                    for hc in range(NH):
                        nc.tensor.matmul(
                            st["scores2"][:, i, :],
                            energyT[hc][:, j * 128 : (j + 1) * 128],
                            v2[:, hc, :],
                            start=(hc == 0),
                            stop=(hc == NH - 1),
                            skip_group_check=True,
                        )
                nc.scalar.activation(
                    st["p2"][:, sg * 4 : (sg + 1) * 4, :],
                    st["scores2"][:, sg * 4 : (sg + 1) * 4, 0:1].broadcast_to(
                        (128, 4, 2)
                    ),
                    mybir.ActivationFunctionType.Exp,
                    accum_out=st["colsum4"][:, sg : sg + 1],
                )

            def stage2(g):  # MM3 (exp of g finished ~one step ago)
                b, sg = groups[g]
                st = state[b]
                if "mm3" in ablate:
                    if sg == 0:
                        nc.vector.memset(st["ctx_acc"], 0.5)
                    return
                e_nat = gstate[g]["e_nat"]
                ctx_sg = ps_mm1.tile([128, NF, 2], F32, tag="mm1")
                for fc in range(NF):
                    for j in range(4):
                        i = sg * 4 + j
                        nc.tensor.matmul(
                            ctx_sg[:, fc, :],
                            e_nat[j][:, fc * 128 : (fc + 1) * 128],
                            st["p2"][:, i, :],
                            start=(j == 0),
                            stop=(j == 3),
                            skip_group_check=True,
                        )
                if sg == 0:
                    nc.vector.tensor_copy(st["ctx_acc"], ctx_sg[:, :, 0])
                else:
                    nc.vector.tensor_add(st["ctx_acc"], st["ctx_acc"], ctx_sg[:, :, 0])
                del gstate[g]

                if sg == NSG - 1:  # batch tail: Z, 1/Z, scale, store
                    for s2 in range(NSG):
                        nc.tensor.matmul(
                            st["z2"],
                            ones_bcast,
                            st["colsum4"][:, s2 : s2 + 1],
                            start=(s2 == 0),
                            stop=(s2 == NSG - 1),
                            skip_group_check=True,
                        )
                    rz2 = small_pool.tile([128, 1], F32, tag="rz")
                    nc.vector.reciprocal(rz2, st["z2"])
                    ctx_sb = small_pool.tile([128, NF], F32, tag="ctx")
                    nc.vector.tensor_scalar(
                        ctx_sb,
                        st["ctx_acc"],
                        rz2,
                        2.0,
                        op0=mybir.AluOpType.mult,
                        op1=mybir.AluOpType.mult,
                    )
                    nc.sync.dma_start(
                        out=out_d[b, :].rearrange("(c p) -> p c", p=128), in_=ctx_sb
                    )

            rep_ctx = tc.For_i(0, reps, 1) if reps > 1 else contextlib.nullcontext()
            with rep_ctx:
                for g in range(NG + 2):
                    if g < NG:
                        stage0(g)
                    if 1 <= g <= NG:
                        stage1(g - 1)
                    if g >= 2:
                        stage2(g - 2)

    nc.finalize()
    return nc


_CACHE = {}


def _get_nc(fast=True, reps=1, ablate=()):
    key = (fast, reps, tuple(ablate))
    if key not in _CACHE:
        _CACHE[key] = _build(fast=fast, reps=reps, ablate=tuple(ablate))
    return _CACHE[key]


def _make_in_maps(hidden, encoder_outputs, W, b, v):
    hidden = np.ascontiguousarray(hidden, dtype=np.float32)
    enc = np.ascontiguousarray(encoder_outputs, dtype=np.float32)
    wt = np.ascontiguousarray(np.asarray(W, dtype=np.float32).T)
    bias = np.ascontiguousarray(b, dtype=np.float32)
    vv = np.ascontiguousarray(v, dtype=np.float32)
    in_maps = []
    for c in range(N_CORES):
        sl = slice(c * B_LOC, (c + 1) * B_LOC)
        in_maps.append(
            {
                "hidden": hidden[sl],
                "enc": enc[sl],
                "wt": wt,
                "bias_in": bias,
                "v_in": vv,
            }
        )
    return in_maps


def _execute(hidden, encoder_outputs, W, b, v, fast=True, **run_kwargs):
    nc = _get_nc(fast)
    in_maps = _make_in_maps(hidden, encoder_outputs, W, b, v)
    res = run_bass_kernel_spmd(nc, in_maps, list(range(N_CORES)), **run_kwargs)
    out = np.concatenate([r["out"] for r in res.results], axis=0)
    return out, res


def kernel(hidden, encoder_outputs, W, b, v):
    out, _ = _execute(hidden, encoder_outputs, W, b, v, fast=True)
    return out


# revision 22
# speedup vs baseline: 1.2630x; 1.2630x over previous
"""Bahdanau-style attention kernel for Trainium2 (8 NeuronCores, SPMD).

Computation (per batch element b):
    q[b]      = hidden[b] @ W1.T                          # [H], W1 = W[:, :2H]
    pre[b,s]  = enc[b,s] @ W2.T + q[b] + bias             # [S, H], W2 = W[:, 2H:]
    energy    = tanh(pre)                                 # [S, H]
    scores    = energy @ v                                # [S]
    attn      = softmax(scores)                           # [S]
    ctx[b]    = enc[b].T @ attn                           # [2H]

Sharding: data-parallel over batch, 4 batches per core, W/b/v replicated.

Layout strategy per batch (per core):
  - enc loaded naturally as 16 tiles e_nat[i] = [s=128, f=1024] (s on partitions).
  - PE transposes 128x128 blocks -> eT tiles [f=128, s=512] for the main matmul
    (contraction over f needs f on partitions).
  - MM1: psum[h=128, s=512] = W2T[f,h].T @ eT[f,s], accumulated over 8 f-chunks.
  - ACT applies tanh with per-partition bias (q[b]+bias)[h-chunk] -> energyT sbuf.
  - MM2: duplicated score column [s=128,2] = energyT[h,s-chunk].T @ v2[h,2],
    accumulated over h-chunks (f32r needs lhsT free dim 128 and even moving N).
  - softmax without max-subtraction (scores bounded by sum|v| ~ 20; exp safe in
    f32): p2 = exp(scores) duplicated, accum_out gives per-partition sums,
    Z broadcast to all partitions via ones matmul.
  - MM3: ctx column pair [f=128,2] = e_nat[i][:,fc].T @ p2[:,i,:], accumulated
    over s-chunks with a single start=True (per-byte has_written lets the 8
    fc-groups share one PSUM bank when only the first matmul starts).
  - scale by 1/Z on DVE, DMA out (column layout -> strided store).

float32r (reduced-precision fp32 matmul, ~4x PE throughput, measured end-to-end
rel err ~8e-5) is used on all heavy matmul paths when fast=True.
"""

import contextlib
import sys

sys.path.insert(0, "/opt/trn_rl_repo")

import numpy as np

import concourse.bass as bass
import concourse.tile as tile
from concourse import bacc, mybir
from concourse.bass_utils import run_bass_kernel_spmd
from concourse.masks import make_identity

F32 = mybir.dt.float32

N_CORES = 8
B = 32
B_LOC = B // N_CORES  # 4 batches per core
S = 2048
H = 512
F = 1024  # 2H = encoder feature dim
NS = S // 128  # 16 s-chunks
NF = F // 128  # 8 f-chunks
NH = H // 128  # 4 h-chunks
NSG = S // 512  # 4 s-groups of 512


def _build(fast=True, reps=1, ablate=()):
    MDT = mybir.dt.float32r if fast else F32
    nc = bacc.Bacc(None, target_bir_lowering=False)

    hid_d = nc.dram_tensor("hidden", [B_LOC, 2 * H], F32, kind="ExternalInput")
    enc_d = nc.dram_tensor("enc", [B_LOC, S, F], MDT, kind="ExternalInput")
    # W.T host-prepared: [2048, 512]; rows 0:1024 = W1.T, 1024:2048 = W2.T
    wt_d = nc.dram_tensor("wt", [4 * H, H], MDT, kind="ExternalInput")
    bias_d = nc.dram_tensor("bias_in", [H], F32, kind="ExternalInput")
    v_d = nc.dram_tensor("v_in", [H], F32, kind="ExternalInput")
    out_d = nc.dram_tensor("out", [B_LOC, F], F32, kind="ExternalOutput")

    with tile.TileContext(nc) as tc:
        with (
            tc.tile_pool(name="singles", bufs=1) as singles,
            tc.tile_pool(name="enat", bufs=22) as enat_pool,
            tc.tile_pool(name="et", bufs=16) as et_pool,
            tc.tile_pool(name="energy", bufs=12) as energy_pool,
            tc.tile_pool(name="small", bufs=4) as small_pool,
            tc.tile_pool(name="pbc", bufs=12) as pbc_pool,
            tc.tile_pool(name="ps_et", bufs=2, space="PSUM") as ps_et,
            tc.tile_pool(name="ps_mm1", bufs=3, space="PSUM") as ps_mm1,
            tc.tile_pool(name="ps_small", bufs=1, space="PSUM") as ps_small,
            tc.tile_pool(name="ps_ctxp", bufs=1, space="PSUM") as ps_ctxp,
        ):
            # ---------------- prologue (once per core) ----------------
            ident = singles.tile([128, 128], F32)
            make_identity(nc, ident)
            if fast:
                ident_r = singles.tile([128, 128], MDT)
                nc.vector.tensor_copy(ident_r, ident)
            else:
                ident_r = ident
            ones_bcast = singles.tile([128, 128], F32)
            nc.vector.memset(ones_bcast, 1.0)

            wt_sb = singles.tile([128, 4 * H // 128, H], MDT)
            nc.sync.dma_start(
                out=wt_sb, in_=wt_d.rearrange("(c p) n -> p c n", p=128)
            )
            v_cols = singles.tile([128, NH], F32)
            nc.sync.dma_start(out=v_cols, in_=v_d.rearrange("(c p) -> p c", p=128))
            bias_cols = singles.tile([128, NH], F32)
            nc.sync.dma_start(
                out=bias_cols, in_=bias_d.rearrange("(c p) -> p c", p=128)
            )
            # duplicated v columns for f32r MM2 (moving operand must be even-width)
            v2 = singles.tile([128, NH, 2], MDT)
            for hc in range(NH):
                nc.scalar.activation(
                    v2[:, hc, :],
                    v_cols[:, hc : hc + 1].broadcast_to((128, 2)),
                    mybir.ActivationFunctionType.Copy,
                )

            # hidden -> hT: column (c*B_LOC + b) = hidden[b, c*128:(c+1)*128]
            hid_sb = singles.tile([B_LOC, 2 * H], F32)
            nc.sync.dma_start(out=hid_sb, in_=hid_d[:, :])
            hT_ps = ps_et.tile([128, 2 * H // 128 * B_LOC], F32, tag="et")
            for c in range(2 * H // 128):
                nc.tensor.transpose(
                    hT_ps[:, c * B_LOC : (c + 1) * B_LOC],
                    hid_sb[:, c * 128 : (c + 1) * 128],
                    ident[:B_LOC, :B_LOC],
                )
            hT_sb = singles.tile([128, 2 * H // 128 * B_LOC], F32)
            nc.scalar.activation(hT_sb, hT_ps, mybir.ActivationFunctionType.Copy)

            # q = hidden @ W1.T (plain fp32, tiny M=4)
            q_ps = ps_mm1.tile([B_LOC, H], F32, tag="mm1")
            for c in range(2 * H // 128):
                nc.tensor.matmul(
                    q_ps,
                    hT_sb[:, c * B_LOC : (c + 1) * B_LOC],
                    wt_sb[:, c, :].bitcast(F32),
                    start=(c == 0),
                    stop=(c == 2 * H // 128 - 1),
                )
            q_sb = singles.tile([B_LOC, H], F32)
            nc.scalar.activation(q_sb, q_ps, mybir.ActivationFunctionType.Copy)

            # qT columns; bias_q[:, hc*B_LOC + b] = q[b, hc-chunk] + bias[hc-chunk]
            qT_ps = ps_et.tile([128, NH * B_LOC], F32, tag="et")
            for hc in range(NH):
                nc.tensor.transpose(
                    qT_ps[:, hc * B_LOC : (hc + 1) * B_LOC],
                    q_sb[:, hc * 128 : (hc + 1) * 128],
                    ident[:B_LOC, :B_LOC],
                )
            bias_q = singles.tile([128, NH * B_LOC], F32)
            for hc in range(NH):
                nc.vector.tensor_scalar_add(
                    bias_q[:, hc * B_LOC : (hc + 1) * B_LOC],
                    qT_ps[:, hc * B_LOC : (hc + 1) * B_LOC],
                    bias_cols[:, hc : hc + 1],
                )

            # ------------- software-pipelined per-s-group stream -------------
            # Global groups g = (b, sg). PE is in-order, so dependent matmuls
            # are emitted LATE: MM2(g) one step after its tanh, MM3(g) two
            # steps after its exp. While ACT computes exp(g), PE is busy with
            # transposes/MM1 of g+1 and MM3 of g-1 -- no cross-engine stalls.
            groups = [(b, sg) for b in range(B_LOC) for sg in range(NSG)]
            NG = len(groups)
            state = {}  # per-batch tiles
            gstate = {}  # per-group tiles

            def stage0(g):  # DMA + transposes + MM1 + tanh issue
                b, sg = groups[g]
                if sg == 0:
                    sps = ps_small.tile([128, 128], F32, tag="sps")
                    state[b] = dict(
                        sps=sps,
                        scores2=sps[:, 0 : 2 * NS].rearrange(
                            "p (i two) -> p i two", two=2
                        ),
                        z2=sps[:, 2 * NS : 2 * NS + 1],
                        colsum=small_pool.tile(
                            [128, NS], F32, tag="colsum", name="colsum"
                        ),
                        ctx_pad=ps_ctxp.tile(
                            [128, 2, 512], F32, tag="ctxp", name="ctx_pad"
                        ),
                    )
                st = state[b]
                e_nat = []
                for j in range(4):
                    i = sg * 4 + j
                    t = enat_pool.tile([128, F], MDT, tag="enat")
                    if "dma" not in ablate:
                        nc.sync.dma_start(
                            out=t, in_=enc_d[b, i * 128 : (i + 1) * 128, :]
                        )
                    e_nat.append(t)

                if "trans" in ablate:
                    et_sb = [wt_sb[:, fc, :] for fc in range(NF)]
                else:
                    et_sb = []
                    for fc in range(NF):
                        et_ps_t = ps_et.tile([128, 512], MDT, tag="et")
                        for j in range(4):
                            nc.tensor.transpose(
                                et_ps_t[:, j * 128 : (j + 1) * 128],
                                e_nat[j][:, fc * 128 : (fc + 1) * 128],
                                ident_r,
                            )
                        t = et_pool.tile([128, 512], MDT, tag="et")
                        nc.vector.tensor_copy(t, et_ps_t)
                        et_sb.append(t)

                energyT = []
                for hc in range(NH):
                    mm1 = ps_mm1.tile([128, 512], F32, tag="mm1")
                    for fc in range(NF):
                        nc.tensor.matmul(
                            mm1,
                            wt_sb[:, NF + fc, hc * 128 : (hc + 1) * 128],
                            et_sb[fc],
                            start=(fc == 0),
                            stop=(fc == NF - 1),
                        )
                    en = energy_pool.tile([128, 512], MDT, tag="energy")
                    nc.scalar.activation(
                        en,
                        mm1,
                        mybir.ActivationFunctionType.Tanh,
                        bias=bias_q[:, hc * B_LOC + b : hc * B_LOC + b + 1],
                    )
                    energyT.append(en)
                gstate[g] = dict(e_nat=e_nat, energyT=energyT)

            def stage1(g):  # MM2 + exp issue (tanh of g finished during g+1's MM1)
                b, sg = groups[g]
                st = state[b]
                energyT = gstate[g]["energyT"]
                for j in range(4):
                    i = sg * 4 + j
                    for hc in range(NH):
                        nc.tensor.matmul(
                            st["scores2"][:, i, :],
                            energyT[hc][:, j * 128 : (j + 1) * 128],
                            v2[:, hc, :],
                            start=(hc == 0),
                            stop=(hc == NH - 1),
                            skip_group_check=True,
                        )
                # exp broadcast into 128 columns: p_bcast[i][s, m] = p(sg*4*128+j*128+s)
                # for every m -- lets MM3 run as p_bcast.T @ e_nat with N=512.
                # accum_out gives 128*p per partition (summed over identical cols).
                pbs = []
                for j in range(4):
                    i = sg * 4 + j
                    pb = pbc_pool.tile([128, 128], MDT, tag="pb", name="pb")
                    nc.scalar.activation(
                        pb,
                        st["scores2"][:, i, 0:1].broadcast_to((128, 128)),
                        mybir.ActivationFunctionType.Exp,
                        accum_out=st["colsum"][:, i : i + 1],
                    )
                    pbs.append(pb)
                gstate[g]["pb"] = pbs

            def stage2(g):  # MM3 (exp of g finished ~one step ago)
                b, sg = groups[g]
                st = state[b]
                e_nat = gstate[g]["e_nat"]
                pbs = gstate[g]["pb"]
                if "mm3" not in ablate:
                    # ctx_pad[:, half, :]: all 128 partitions hold the same ctx
                    # row; halves live in different PSUM banks so their
                    # accumulation groups are independent.
                    for j in range(4):
                        for half in range(2):
                            nc.tensor.matmul(
                                st["ctx_pad"][:, half, :],
                                pbs[j],
                                e_nat[j][:, half * 512 : (half + 1) * 512],
                                start=(sg == 0 and j == 0),
                                stop=(sg == NSG - 1 and j == 3),
                                skip_group_check=True,
                            )
                del gstate[g]

                if sg == NSG - 1:  # batch tail: Z, 1/Z, scale, store
                    cs1 = small_pool.tile([128, 1], F32, tag="cs1")
                    nc.vector.reduce_sum(
                        out=cs1, in_=st["colsum"], axis=mybir.AxisListType.X
                    )
                    nc.tensor.matmul(
                        st["z2"], ones_bcast, cs1, skip_group_check=True
                    )
                    rz2 = small_pool.tile([128, 1], F32, tag="rz")
                    nc.vector.reciprocal(rz2, st["z2"])
                    # rz2 = 1/(128*Z); ctx needs *128
                    ctx_sb = small_pool.tile([1, F], F32, tag="ctx")
                    nc.vector.tensor_scalar(
                        ctx_sb,
                        st["ctx_pad"][0:1, :, :].rearrange("p a b -> p (a b)"),
                        rz2[0:1, :],
                        128.0,
                        op0=mybir.AluOpType.mult,
                        op1=mybir.AluOpType.mult,
                    )
                    nc.sync.dma_start(out=out_d[b : b + 1, :], in_=ctx_sb)

            rep_ctx = tc.For_i(0, reps, 1) if reps > 1 else contextlib.nullcontext()
            with rep_ctx:
                for g in range(NG + 2):
                    if g < NG:
                        stage0(g)
                    if 1 <= g <= NG:
                        stage1(g - 1)
                    if g >= 2:
                        stage2(g - 2)

    nc.finalize()
    return nc


_CACHE = {}


def _get_nc(fast=True, reps=1, ablate=()):
    key = (fast, reps, tuple(ablate))
    if key not in _CACHE:
        _CACHE[key] = _build(fast=fast, reps=reps, ablate=tuple(ablate))
    return _CACHE[key]


def _make_in_maps(hidden, encoder_outputs, W, b, v):
    hidden = np.ascontiguousarray(hidden, dtype=np.float32)
    enc = np.ascontiguousarray(encoder_outputs, dtype=np.float32)
    wt = np.ascontiguousarray(np.asarray(W, dtype=np.float32).T)
    bias = np.ascontiguousarray(b, dtype=np.float32)
    vv = np.ascontiguousarray(v, dtype=np.float32)
    in_maps = []
    for c in range(N_CORES):
        sl = slice(c * B_LOC, (c + 1) * B_LOC)
        in_maps.append(
            {
                "hidden": hidden[sl],
                "enc": enc[sl],
                "wt": wt,
                "bias_in": bias,
                "v_in": vv,
            }
        )
    return in_maps


def _execute(hidden, encoder_outputs, W, b, v, fast=True, **run_kwargs):
    nc = _get_nc(fast)
    in_maps = _make_in_maps(hidden, encoder_outputs, W, b, v)
    res = run_bass_kernel_spmd(nc, in_maps, list(range(N_CORES)), **run_kwargs)
    out = np.concatenate([r["out"] for r in res.results], axis=0)
    return out, res


def kernel(hidden, encoder_outputs, W, b, v):
    out, _ = _execute(hidden, encoder_outputs, W, b, v, fast=True)
    return out
